# revision 1
# baseline (speedup 1.0000x reference)
"""Multi-head attention (GQA, RoPE, causal) Trainium2 Bass kernel, 8-core SPMD.

Sharding: tensor-parallel over heads. Core c owns q-heads [4c, 4c+4) and kv-head c
(wq/wk/wv column-sharded, wo row-sharded). Each core computes a full-shape
[S, DIM] partial of the output (its heads' contribution through wo); the host
sums the 8 partials.

Per-core dataflow (all matmuls fp32r: full PE rate at free-dim >= 256):
  A) load x tiles [128s, 2048d] -> PE-transpose to xT [d-chunks, s] ->
     project QT [4h*64, s], KT [64, s] (both RoPE'd), V [s, 64+ones-col]
  B) per (q-block 512, head, key-chunk 128):
       scoresT[k, q] = KT_chunk.T @ QT  (K=64; even/odd heads sit at partition
       bases 0/64 so pairs run concurrently in separate PE row-groups)
       expT = exp(scoresT/8) [ACT, reads PSUM], diag-masked by 4 static patterns
       out_unnorm/denom accumulate via [V | 1] lhsT (denominator is free)
     normalize with DVE reciprocal + gpsimd partition-broadcast
  C) out_partial = attn_outT.T @ wo_shard
"""

import numpy as np

DIM = 2048
NH = 32
NKV = 8
HD = 64
S = 2048
NCORES = 8
HPC = NH // NCORES  # 4 q-heads per core
SBW = 256           # phase-A seq block width
QB = 512            # phase-B q block width
NSB = S // SBW
NQB = S // QB
NKC = S // 128


def _install_walrus_workarounds():
    """The walrus build in this container accepts at most ONE sync-wait per
    instruction; Tile emits up to three. Rewrite the BIR before compiling:
    hoist extra waits onto preceding NoOps on the same engine (the engine
    executes serially, so waiting on a prior NoOp is equivalent)."""
    import orjson
    import concourse.bass2jax as b2j
    import concourse.bass_utils as bu
    if getattr(b2j, "_ant_wait_split_installed", False):
        return
    orig_compile = bu.compile_bir_kernel
    ctr = [0]

    def _split(bir_bytes):
        d = orjson.loads(bir_bytes)
        changed = False
        for fn in d.get("functions", []):
            for blk in fn.get("blocks", []):
                out = []
                for ins in blk.get("instructions", []):
                    si = ins.get("sync_info")
                    waits = (si or {}).get("on_wait") or []
                    if len(waits) > 1:
                        changed = True
                        for w in waits[:-1]:
                            ctr[0] += 1
                            out.append({
                                "debug": ins.get("debug"),
                                "engine": ins.get("engine"),
                                "ins": [], "outs": [],
                                "name": f"I-wsplit-{ctr[0]}",
                                "opcode": "NoOp",
                                "sync_info": {"on_update": [], "on_wait": [w]},
                                "text_hint": "wsplit",
                            })
                        si["on_wait"] = [waits[-1]]
                    out.append(ins)
                blk["instructions"] = out
        return orjson.dumps(d) if changed else bir_bytes

    def patched_compile_bir_kernel(bir_json, tmpdir, neff_name="file.neff"):
        if isinstance(bir_json, (bytes, bytearray)):
            bir_json = _split(bytes(bir_json))
        return orig_compile(bir_json, tmpdir, neff_name)

    b2j.compile_bir_kernel = patched_compile_bir_kernel
    bu.compile_bass_kernel = lambda nc, tmpdir, neff_name="file.neff": (
        patched_compile_bir_kernel(nc.to_json_bytes(), tmpdir, neff_name))
    b2j._ant_wait_split_installed = True


def _build(causal, use_fullmask):
    from contextlib import ExitStack
    import concourse.bass as bass
    import concourse.mybir as mybir
    import concourse.tile as tile
    from concourse.masks import make_identity

    f32 = mybir.dt.float32
    f32r = mybir.dt.float32r
    Exp = mybir.ActivationFunctionType.Exp

    nc = bass.Bass()
    x_d = nc.dram_tensor("x2d", [S, DIM], f32r, kind="ExternalInput")
    wq_d = nc.dram_tensor("wq_sh", [DIM, HPC * HD], f32r, kind="ExternalInput")
    wkv_d = nc.dram_tensor("wkv_sh", [DIM, 2 * HD], f32r, kind="ExternalInput")
    wo_d = nc.dram_tensor("wo_sh", [HPC * HD, DIM], f32, kind="ExternalInput")
    cos_d = nc.dram_tensor("cos128", [128, S], f32, kind="ExternalInput")
    sin_d = nc.dram_tensor("sin128", [128, S], f32, kind="ExternalInput")
    pm_d = nc.dram_tensor("pmat", [128, 128], f32, kind="ExternalInput")
    if causal:
        dm_d = nc.dram_tensor("dmask", [128, 4 * QB], f32, kind="ExternalInput")
    if use_fullmask:
        fm_d = nc.dram_tensor("fmaskT", [S, S], f32, kind="ExternalInput")
    out_d = nc.dram_tensor("outp", [S, DIM], f32, kind="ExternalOutput")

    with tile.TileContext(nc) as tc, ExitStack() as ctx:
        const = ctx.enter_context(tc.tile_pool(name="const", bufs=1))
        persist = ctx.enter_context(tc.tile_pool(name="persist", bufs=1))
        px = ctx.enter_context(tc.tile_pool(name="px", bufs=2))
        pxt = ctx.enter_context(tc.tile_pool(name="pxt", bufs=1))
        prope = ctx.enter_context(tc.tile_pool(name="prope", bufs=2))
        pexp = ctx.enter_context(tc.tile_pool(name="pexp", bufs=6))
        pnorm = ctx.enter_context(tc.tile_pool(name="pnorm", bufs=2))
        pout = ctx.enter_context(tc.tile_pool(name="pout", bufs=4))
        pdram = ctx.enter_context(tc.tile_pool(name="pdram", bufs=2, space="DRAM"))
        pp_tp = ctx.enter_context(tc.tile_pool(name="pp_tp", bufs=2, space="PSUM"))
        pp_pj = ctx.enter_context(tc.tile_pool(name="pp_pj", bufs=2, space="PSUM"))
        pp_ps = ctx.enter_context(tc.tile_pool(name="pp_ps", bufs=2, space="PSUM"))
        pp_po = ctx.enter_context(tc.tile_pool(name="pp_po", bufs=2, space="PSUM"))

        # Const loads all on SWDGE (gpsimd) so the SP/HWDGE queue starts
        # streaming x immediately; weights first (needed earliest), wq split
        # so the m=0 projection can start after ~half the wq bytes land.
        ident = const.tile([128, 128], f32)
        make_identity(nc, ident)
        ident_r = const.tile([128, 128], f32r)
        nc.gpsimd.dma_start(out=ident_r, in_=ident[:, :])
        wq_sb = const.tile([128, NKC, HPC * HD], f32r)
        wq_r = wq_d[:, :].rearrange("(c p) m -> p c m", p=128)
        nc.scalar.dma_start(out=wq_sb[:, :, 0:128], in_=wq_r[:, :, 0:128])
        nc.scalar.dma_start(out=wq_sb[:, :, 128:256], in_=wq_r[:, :, 128:256])
        wkv_sb = const.tile([128, NKC, 2 * HD], f32r)
        nc.scalar.dma_start(out=wkv_sb, in_=wkv_d[:, :].rearrange("(c p) m -> p c m", p=128))
        pm_sb = const.tile([128, 128], f32r)
        nc.gpsimd.dma_start(out=pm_sb, in_=pm_d[:, :])
        cos_sb = const.tile([128, S], f32)
        nc.gpsimd.dma_start(out=cos_sb, in_=cos_d[:, :])
        sin_sb = const.tile([128, S], f32)
        nc.gpsimd.dma_start(out=sin_sb, in_=sin_d[:, :])
        if causal:
            dm_sb = const.tile([128, 4, QB], f32r)
            nc.gpsimd.dma_start(out=dm_sb, in_=dm_d[:, :])
        wo_sb = const.tile([128, 2, DIM], f32r)
        nc.gpsimd.dma_start(out=wo_sb, in_=wo_d[:, :].rearrange("(c p) n -> p c n", p=128))

        QT = persist.tile([128, 2, S], f32r)      # q-head pair chunks, RoPE'd, [dim, seq]
        KT2 = persist.tile([128, S], f32r)        # kv head [64 dims, seq], duplicated rows 64-127
        Vplus = persist.tile([128, NKC, HD + 1], f32r)  # [key-chunk part, chunk, 64 V dims + ones]
        AO = persist.tile([128, 2, S], f32r)      # attention outT [4h*64, seq]
        ones_sb = const.tile([128, NKC], f32)
        nc.vector.memset(ones_sb, 1.0)
        nc.vector.tensor_copy(out=Vplus[:, :, HD], in_=ones_sb)

        def rope(pp_src, r0, r1, dest_ap, s0, width):
            # dest = src*cos + (P.T @ src)*sin; P carries the rotate-half
            # permutation and its signs (block-diagonal per 64-row head group).
            raw = prope.tile([128, SBW], f32r, tag="raw")
            t1 = prope.tile([128, SBW], f32, tag="t1")
            t2 = prope.tile([128, SBW], f32, tag="t2")
            # copy the full 128 rows (fp32r matmuls reject output col-groups at
            # base 64, so the perm matmul always runs full-array at (0,0);
            # extra rows are computed and ignored)
            nc.vector.tensor_copy(out=raw[:, :width], in_=pp_src[:, :width])
            nc.vector.tensor_mul(out=t1[r0:r1, :width], in0=pp_src[r0:r1, :width],
                                 in1=cos_sb[r0:r1, s0:s0 + width])
            pperm = pp_tp.tile([128, SBW], f32, tag="tp")
            nc.tensor.matmul(pperm[:, :width], pm_sb[:, :],
                             raw[:, :width], start=True, stop=True)
            nc.vector.tensor_mul(out=t2[r0:r1, :width], in0=pperm[r0:r1, :width],
                                 in1=sin_sb[r0:r1, s0:s0 + width])
            # final add on the (idle) gpsimd engine — all-SBUF operands
            nc.gpsimd.tensor_add(out=dest_ap, in0=t1[r0:r1, :width], in1=t2[r0:r1, :width])

        # psum->sbuf copies balanced across DVE and ACT (both ~1 elem/cyc/lane
        # when an operand is in PSUM)
        cp_ctr = [0]

        def copy_ps(out_ap, in_ap):
            cp_ctr[0] += 1
            if cp_ctr[0] % 3 != 0:
                nc.vector.tensor_copy(out=out_ap, in_=in_ap)
            else:
                nc.scalar.copy(out=out_ap, in_=in_ap)

        # ---- Phase A: transpose x, project, RoPE ----
        def emit_A(sb):
            s0 = sb * SBW
            xT = pxt.tile([128, NKC, SBW], f32r)
            for t in range(SBW // 128):
                xin = px.tile([128, DIM], f32r)
                nc.sync.dma_start(out=xin, in_=x_d[s0 + t * 128:s0 + (t + 1) * 128, :])
                for g in range(4):
                    pt = pp_tp.tile([128, 4, 128], f32r, tag="tp")
                    for j in range(4):
                        cc = 4 * g + j
                        nc.tensor.transpose(pt[:, j, :], xin[:, cc * 128:(cc + 1) * 128], ident_r)
                    copy_ps(xT[:, 4 * g:4 * g + 4, t * 128:(t + 1) * 128], pt)
            for m in range(3):
                pp = pp_pj.tile([128, SBW], f32, tag="pj")
                for k in range(NKC):
                    if m == 0:
                        lhs = wq_sb[:, k, 0:128]
                    elif m == 1:
                        lhs = wq_sb[:, k, 128:256]
                    else:
                        lhs = wkv_sb[:, k, :]
                    nc.tensor.matmul(pp[:, :], lhs, xT[:, k, :],
                                     start=(k == 0), stop=(k == NKC - 1))
                if m < 2:
                    rope(pp, 0, 128, QT[:, m, s0:s0 + SBW], s0, SBW)
                else:
                    vt = prope.tile([64, SBW], f32r, tag="vt")
                    nc.vector.tensor_copy(out=vt, in_=pp[0:64, :])
                    rope(pp, 64, 128, KT2[64:128, s0:s0 + SBW], s0, SBW)
                    nc.gpsimd.dma_start(out=KT2[0:64, s0:s0 + SBW], in_=KT2[64:128, s0:s0 + SBW])
                    for j in range(SBW // 128):
                        pv = pp_tp.tile([128, 64], f32r, tag="tp")
                        nc.tensor.transpose(pv, vt[:, j * 128:(j + 1) * 128], ident_r[0:64, 0:64])
                        nc.vector.tensor_copy(out=Vplus[:, sb * (SBW // 128) + j, 0:HD], in_=pv)

        # ---- Phase B: attention ----
        def emit_B(qb):
            q0 = qb * QB
            for h in range(HPC):
                c = h // 2
                pb = (h % 2) * 64
                po = pp_po.tile([128, QB], f32, tag="po")
                kc_hi = (4 * qb + 4) if causal else NKC
                for kc in range(kc_hi):
                    # causal diag blocks: only columns >= off are (partially)
                    # unmasked; off capped at 256 to keep fp32r full-rate.
                    dc = kc - 4 * qb if causal else -1
                    off = min(dc * 128, 256) if dc > 0 else 0
                    w = QB - off
                    ps = pp_ps.tile([128, QB], f32, tag="ps")
                    nc.tensor.matmul(ps[:, off:], KT2[pb:pb + 64, kc * 128:(kc + 1) * 128],
                                     QT[pb:pb + 64, c, q0 + off:q0 + QB],
                                     start=True, stop=True)
                    et = pexp.tile([128, QB], f32r, tag="et")
                    nc.scalar.activation(out=et[:, off:], in_=ps[:, off:],
                                         func=Exp, scale=0.125)
                    if causal and dc >= 0:
                        # residual boundary pattern inside the slice
                        cp = dc - off // 128
                        mw = 128 * (cp + 1)
                        nc.vector.tensor_mul(out=et[:, off:off + mw],
                                             in0=et[:, off:off + mw],
                                             in1=dm_sb[:, cp, 0:mw])
                    if use_fullmask:
                        fmt = pexp.tile([128, QB], f32r, tag="fmt")
                        nc.gpsimd.dma_start(out=fmt, in_=fm_d[kc * 128:(kc + 1) * 128, q0:q0 + QB])
                        nc.vector.tensor_mul(out=et, in0=et, in1=fmt)
                    nc.tensor.matmul(po[0:HD + 1, off:], Vplus[:, kc, :], et[:, off:],
                                     start=(kc == 0), stop=(kc == kc_hi - 1))
                rec = pnorm.tile([1, QB], f32, tag="rec")
                nc.vector.reciprocal(out=rec, in_=po[HD:HD + 1, :])
                drec = pdram.tile([1, QB], f32, tag="drec")
                nc.gpsimd.dma_start(out=drec, in_=rec)
                bc = pnorm.tile([64, QB], f32, tag="bc")
                dfull = drec[:, :]
                rbc = bass.AP(tensor=dfull.tensor, offset=dfull.offset,
                              ap=[[0, 64], dfull.ap[1]])
                nc.gpsimd.dma_start(out=bc, in_=rbc)
                if pb == 0:
                    nc.vector.tensor_mul(out=AO[0:64, c, q0:q0 + QB], in0=po[0:HD, :], in1=bc)
                else:
                    nt = pnorm.tile([64, QB], f32r, tag="nt")
                    nc.vector.tensor_mul(out=nt, in0=po[0:HD, :], in1=bc)
                    nc.gpsimd.dma_start(out=AO[64:128, c, q0:q0 + QB], in_=nt)

        # ---- Phase C: output projection (partial over this core's heads) ----
        def emit_C(qb):
            for mt in range(4 * qb, 4 * qb + 4):
                r0 = mt * 128
                for nb in range(DIM // 512):
                    pcb = pp_pj.tile([128, 512], f32, tag="pj")
                    for cc in range(2):
                        nc.tensor.matmul(pcb[:, :], AO[:, cc, r0:r0 + 128],
                                         wo_sb[:, cc, nb * 512:(nb + 1) * 512],
                                         start=(cc == 0), stop=(cc == 1))
                    osb = pout.tile([128, 512], f32, tag="osb")
                    copy_ps(osb, pcb[:, :])
                    nc.sync.dma_start(out=out_d[r0:r0 + 128, nb * 512:(nb + 1) * 512], in_=osb)

        # Interleave phases in emission (priority) order so exp/ACT work from
        # B(qb) overlaps A's PE-heavy transposes/projections, and C trails B.
        # B(qb) depends on A(sb <= 2qb+1); C(qb) on B(qb) for all heads.
        emit_A(0)
        emit_A(1)
        emit_B(0)
        emit_A(2)
        emit_A(3)
        emit_B(1)
        emit_A(4)
        emit_A(5)
        emit_C(0)
        emit_B(2)
        emit_A(6)
        emit_A(7)
        emit_C(1)
        emit_B(3)
        emit_C(2)
        emit_C(3)

    return nc


_NC_CACHE = {}
LAST_RESULTS = None


def _rope_tables():
    # mirror reference float32 arithmetic
    inv = (1.0 / (10000.0 ** (np.arange(0, HD, 2, dtype=np.float32) / np.float32(HD)))).astype(np.float32)
    t = np.arange(S, dtype=np.float32)
    f = np.outer(t, inv).astype(np.float32)          # [S, 32]
    emb = np.concatenate([f, f], axis=1)             # [S, 64]
    cos64 = np.cos(emb).astype(np.float32).T         # [64, S]
    sin64 = np.sin(emb).astype(np.float32).T
    cos128 = np.ascontiguousarray(np.tile(cos64, (2, 1)))
    sin128 = np.ascontiguousarray(np.tile(sin64, (2, 1)))
    return cos128, sin128


def _pmat():
    # P.T @ x applies rotate_half incl. signs: out[j] = -x[2j+1] (j<32),
    # out[32+j] = x[2j]. P[i, j] = coeff of src row i in dst row j.
    p64 = np.zeros((64, 64), np.float32)
    for j in range(32):
        p64[2 * j + 1, j] = -1.0
        p64[2 * j, 32 + j] = 1.0
    pm = np.zeros((128, 128), np.float32)
    pm[:64, :64] = p64
    pm[64:, 64:] = p64
    return np.ascontiguousarray(pm)


def _diag_masks():
    kk = np.arange(128)[:, None]
    qq = np.arange(QB)[None, :]
    pats = [(qq >= kk + 128 * c).astype(np.float32) for c in range(4)]
    return np.ascontiguousarray(np.concatenate(pats, axis=1))  # [128, 4*QB]


def kernel(x, mask, wq, wk, wv, wo):
    global LAST_RESULTS
    import os
    _install_walrus_workarounds()
    from concourse import bass_utils

    x = np.ascontiguousarray(np.asarray(x, dtype=np.float32).reshape(S, DIM))
    wq = np.asarray(wq, dtype=np.float32)
    wk = np.asarray(wk, dtype=np.float32)
    wv = np.asarray(wv, dtype=np.float32)
    wo = np.asarray(wo, dtype=np.float32)
    mb = (np.asarray(mask).reshape(S, S) != 0)

    causal = bool(np.array_equal(mb, np.tril(np.ones((S, S), dtype=bool))))
    allones = bool(mb.all())
    use_fullmask = not (causal or allones)

    key = (causal, use_fullmask)
    if key not in _NC_CACHE:
        _NC_CACHE[key] = _build(causal, use_fullmask)
    nc = _NC_CACHE[key]

    cos128, sin128 = _rope_tables()
    dmask = _diag_masks()
    fmaskT = None
    if use_fullmask:
        fmaskT = np.ascontiguousarray(mb.T.astype(np.float32))

    in_maps = []
    for cidx in range(NCORES):
        h0 = cidx * HPC
        im = {
            "x2d": x,
            "wq_sh": np.ascontiguousarray(wq[:, h0 * HD:(h0 + HPC) * HD]),
            "wkv_sh": np.ascontiguousarray(
                np.concatenate([wv[:, cidx * HD:(cidx + 1) * HD],
                                wk[:, cidx * HD:(cidx + 1) * HD]], axis=1)),
            "wo_sh": np.ascontiguousarray(wo[h0 * HD:(h0 + HPC) * HD, :]),
            "cos128": cos128,
            "sin128": sin128,
            "pmat": _pmat(),
        }
        if causal:
            im["dmask"] = dmask
        if use_fullmask:
            im["fmaskT"] = fmaskT
        in_maps.append(im)

    trace = bool(int(os.environ.get("KERNEL_TRACE", "0")))
    res = bass_utils.run_bass_kernel_spmd(
        nc, in_maps, core_ids=list(range(NCORES)), trace=trace)
    LAST_RESULTS = res

    total = np.zeros((S, DIM), dtype=np.float32)
    for r in res.results:
        total += r["outp"]
    return total.reshape(1, S, DIM)

